# revision 47
# baseline (speedup 1.0000x reference)
"""CrossViewSwapAttention Trainium2 kernel (v2: fp16/bf16 matmul path).

Sharding: the 64 window groups L=X*Y are split across 8 cores (8 groups per
core); each window group's attention is fully local to a core. Host folds the
LayerNorm gains/biases, attn_scale, dh**-0.5, the mean-over-views 1/n and bp
into the weights, rearranges q/k/v to [L, tokens, d] token-major shards (cast
to fp16), and re-assembles the output.

Device program per core (8 groups, pipelined pairwise):
  DMA x (fp16, one DMA per tensor per group-pair) -> bn_stats + combined
  LN stats per pair (DVE) -> LN apply (DVE q / Pool kv, in place, fp16) ->
  PE transposes (fp16, 1cyc/row) -> psum drains (DVE/Pool) ->
  q/k/v projections (fp16 matmuls) -> attention per (head, chunk-pair) with
  double-buffered score psum: scores (fp16) -> exp (ACT -> bf16 pt) ->
  AV + ones-matmul row-sums (bf16, per-head psum accumulation groups) ->
  softmax division (DVE) -> out-proj (fp16) -> view-mean reduce -> +skip.
"""
import numpy as np

HEADS, DH, D = 4, 32, 128
LN_EPS = 1e-5
NCORES = 8
LPC = 8          # L window-groups per core
TQ, TK = 384, 480
QC = 3           # q chunks of 128
KC, KCS = 4, 120  # kv chunks
NPAIR = LPC // 2

_prog = {}


def _build_program():
    import concourse.bass as bass
    import concourse.tile as tile
    from concourse import mybir
    from concourse.tile import ScopedClock

    # -- walrus workaround: split tail-drain sem waits into single-wait NOPs --
    def _drain_and_barrier(self, tick_clock, wait_clock):
        nc = self.nc
        nop0 = nc.sync.nop()
        wait_clock.add_sem_waits(nop0.ins, ScopedClock({None: tick_clock.global_clock}))
        si = nop0.ins.sync_info
        waits = list(si.on_wait) if si is not None else []
        if len(waits) > 1:
            nop0.ins.sync_info = mybir.SyncInfo(on_wait=waits[:1], on_update=list(si.on_update))
            for w in waits[1:]:
                n = nc.sync.nop()
                n.ins.sync_info = mybir.SyncInfo(on_wait=[w], on_update=[])
        nc.sync.drain()
        nc.all_engine_barrier()
        assert self.sems is not None
        popped = nc._tile_sem_poison_stack.pop()
        assert popped is self._sem_poison
        nc.clear_and_free_semaphores(list(self.sems.allocated().values()))
        nc.all_engine_barrier()

    tile.TileContext._drain_and_barrier = _drain_and_barrier

    f32 = mybir.dt.float32
    f16 = mybir.dt.float16
    bf16 = mybir.dt.bfloat16
    nc = bass.Bass()
    d_q = nc.declare_dram_parameter("q", [LPC, TQ, D], f16, isOutput=False)
    d_k = nc.declare_dram_parameter("k", [LPC, TK, D], f16, isOutput=False)
    d_v = nc.declare_dram_parameter("v", [LPC, TK, D], f16, isOutput=False)
    d_skip = nc.declare_dram_parameter("skip", [LPC, 64, D], f32, isOutput=False)
    d_wq = nc.declare_dram_parameter("wq", [D, D], f16, isOutput=False)
    d_wk = nc.declare_dram_parameter("wk", [D, D], f16, isOutput=False)
    d_wv = nc.declare_dram_parameter("wv", [D, D], f16, isOutput=False)
    d_wpx = nc.declare_dram_parameter("wpx", [D, D], f16, isOutput=False)
    d_wpy = nc.declare_dram_parameter("wpy", [D, D], f16, isOutput=False)
    d_cq = nc.declare_dram_parameter("cq", [D, 1], f32, isOutput=False)
    d_idh = nc.declare_dram_parameter("identh", [D, D], f16, isOutput=False)
    d_id32 = nc.declare_dram_parameter("ident32", [D, D], f32, isOutput=False)
    d_out = nc.declare_dram_parameter("out", [LPC, 64, D], f32, isOutput=True)

    X = mybir.AxisListType.X
    SUB, MUL, ADD = mybir.AluOpType.subtract, mybir.AluOpType.mult, mybir.AluOpType.add

    from contextlib import ExitStack
    with tile.TileContext(nc) as tc, ExitStack() as es:
        cst = es.enter_context(tc.tile_pool(name="cst", bufs=1))
        xp = es.enter_context(tc.tile_pool(name="xp", bufs=3))
        st = es.enter_context(tc.tile_pool(name="st", bufs=3))
        sb = es.enter_context(tc.tile_pool(name="sb", bufs=2))
        ptp = es.enter_context(tc.tile_pool(name="ptp", bufs=2))
        outp = es.enter_context(tc.tile_pool(name="outp", bufs=3))
        work = es.enter_context(tc.tile_pool(name="work", bufs=2, space="PSUM"))
        scb = es.enter_context(tc.tile_pool(name="scb", bufs=2, space="PSUM"))
        asb = es.enter_context(tc.tile_pool(name="asb", bufs=1, space="PSUM"))

        # pair-0 input DMAs issue first so group 0's stats are not stuck
        # behind const loads on the HWDGE queue
        pair_tiles = {}

        def emit_load_pair(j):
            xq = xp.tile([128, 2 * QC, D], f16, tag="xq", name=f"xq{j}")
            nc.sync.dma_start(xq[:], d_q[2 * j:2 * j + 2].rearrange("l (c p) d -> p (l c) d", p=128))
            xk = xp.tile([KCS, 2 * KC, D], f16, tag="xk", name=f"xk{j}")
            nc.sync.dma_start(xk[:], d_k[2 * j:2 * j + 2].rearrange("l (c p) d -> p (l c) d", p=KCS))
            xv = xp.tile([KCS, 2 * KC, D], f16, tag="xv", name=f"xv{j}")
            nc.sync.dma_start(xv[:], d_v[2 * j:2 * j + 2].rearrange("l (c p) d -> p (l c) d", p=KCS))
            pair_tiles[j] = dict(xq=xq, xk=xk, xv=xv)

        def emit_load_skip(j):
            skp = outp.tile([64, 2, D], f32, tag="skp", name=f"skp{j}")
            nc.sync.dma_start(skp[:], d_skip[2 * j:2 * j + 2].rearrange("l t d -> t l d"))
            pair_tiles[j]["skp"] = skp

        emit_load_pair(0)

        # constants (transpose identity + projection weights first: they gate
        # the first PE work; the rest can trickle in behind pair-0 stats)
        idh_sb = cst.tile([D, D], f16, tag="idh"); nc.sync.dma_start(idh_sb[:], d_idh[:])
        wq_sb = cst.tile([D, D], f16, tag="wq"); nc.sync.dma_start(wq_sb[:], d_wq[:])
        wk_sb = cst.tile([D, D], f16, tag="wk"); nc.sync.dma_start(wk_sb[:], d_wk[:])
        wv_sb = cst.tile([D, D], f16, tag="wv"); nc.sync.dma_start(wv_sb[:], d_wv[:])
        cq_sb = cst.tile([D, 1], f32, tag="cq"); nc.sync.dma_start(cq_sb[:], d_cq[:])
        wpx_sb = cst.tile([D, D], f16, tag="wpx"); nc.sync.dma_start(wpx_sb[:], d_wpx[:])
        wpy_sb = cst.tile([D, D], f16, tag="wpy"); nc.sync.dma_start(wpy_sb[:], d_wpy[:])
        id32_sb = cst.tile([D, D], f32, tag="id32"); nc.sync.dma_start(id32_sb[:], d_id32[:])

        # per-chunk even/odd bn_stats combine -> mu, rstd (chunk width 128);
        # bn_stats is issued once per chunk-PAIR (free 256) for half the
        # instruction count; its per-chunk 6-col stats land per chunk slot
        def stats_combine(s, P, NCH, eng=None):
            eng = eng or nc.gpsimd
            n = 128
            me, mo = s[:P, :, 1:2].rearrange("p c o -> p (c o)"), s[:P, :, 4:5].rearrange("p c o -> p (c o)")
            m2e, m2o = s[:P, :, 2:3].rearrange("p c o -> p (c o)"), s[:P, :, 5:6].rearrange("p c o -> p (c o)")
            mu = st.tile([P, NCH], f32, tag=f"mu{P}{NCH}")
            rstd = st.tile([P, NCH], f32, tag=f"rstd{P}{NCH}")
            tmp = st.tile([P, NCH], f32, tag=f"tmp{P}{NCH}")
            m2 = st.tile([P, NCH], f32, tag=f"m2{P}{NCH}")
            eng.tensor_tensor(out=tmp[:], in0=me, in1=mo, op=ADD)
            eng.tensor_scalar_mul(mu[:], tmp[:], 0.5)
            eng.tensor_tensor(out=tmp[:], in0=mo, in1=me, op=SUB)
            eng.tensor_tensor(out=tmp[:], in0=tmp[:], in1=tmp[:], op=MUL)
            eng.tensor_scalar_mul(tmp[:], tmp[:], float(n) / 4.0)
            eng.tensor_tensor(out=m2[:], in0=m2e, in1=m2o, op=ADD)
            eng.tensor_tensor(out=m2[:], in0=m2[:], in1=tmp[:], op=ADD)
            # v = var + eps ; r0 = 1/sqrt_approx(v) ; one Newton step for the
            # loose ACT sqrt: r1 = r0*(1.5 - 0.5*v*r0^2)
            vv = st.tile([P, NCH], f32, tag=f"vv{P}{NCH}")
            eng.tensor_scalar(out=vv[:], in0=m2[:], scalar1=1.0 / float(n),
                                    scalar2=float(LN_EPS), op0=MUL, op1=ADD)
            nc.scalar.activation(rstd[:], vv[:], mybir.ActivationFunctionType.Sqrt)
            nc.vector.reciprocal(rstd[:], rstd[:])
            t2 = st.tile([P, NCH], f32, tag=f"t2{P}{NCH}")
            eng.tensor_tensor(out=t2[:], in0=rstd[:], in1=rstd[:], op=MUL)
            eng.tensor_tensor(out=t2[:], in0=t2[:], in1=vv[:], op=MUL)
            eng.tensor_scalar(out=t2[:], in0=t2[:], scalar1=-0.5, scalar2=1.5, op0=MUL, op1=ADD)
            eng.tensor_tensor(out=rstd[:], in0=rstd[:], in1=t2[:], op=MUL)
            return mu, rstd

        pair_tiles = {}

        def emit_load_pair(j):
            xq = xp.tile([128, 2 * QC, D], f16, tag="xq", name=f"xq{j}")
            nc.sync.dma_start(xq[:], d_q[2 * j:2 * j + 2].rearrange("l (c p) d -> p (l c) d", p=128))
            xk = xp.tile([KCS, 2 * KC, D], f16, tag="xk", name=f"xk{j}")
            nc.sync.dma_start(xk[:], d_k[2 * j:2 * j + 2].rearrange("l (c p) d -> p (l c) d", p=KCS))
            xv = xp.tile([KCS, 2 * KC, D], f16, tag="xv", name=f"xv{j}")
            nc.sync.dma_start(xv[:], d_v[2 * j:2 * j + 2].rearrange("l (c p) d -> p (l c) d", p=KCS))
            pair_tiles[j] = dict(xq=xq, xk=xk, xv=xv)

        def emit_load_skip(j):
            skp = outp.tile([64, 2, D], f32, tag="skp", name=f"skp{j}")
            nc.sync.dma_start(skp[:], d_skip[2 * j:2 * j + 2].rearrange("l t d -> t l d"))
            pair_tiles[j]["skp"] = skp

        emit_load_pair(0)

        # constants (transpose identity + projection weights first: they gate
        # the first PE work; the rest can trickle in behind pair-0 stats)
        idh_sb = cst.tile([D, D], f16, tag="idh"); nc.sync.dma_start(idh_sb[:], d_idh[:])
        wq_sb = cst.tile([D, D], f16, tag="wq"); nc.sync.dma_start(wq_sb[:], d_wq[:])
        wk_sb = cst.tile([D, D], f16, tag="wk"); nc.sync.dma_start(wk_sb[:], d_wk[:])
        wv_sb = cst.tile([D, D], f16, tag="wv"); nc.sync.dma_start(wv_sb[:], d_wv[:])
        cq_sb = cst.tile([D, 1], f32, tag="cq"); nc.sync.dma_start(cq_sb[:], d_cq[:])
        wpx_sb = cst.tile([D, D], f16, tag="wpx"); nc.sync.dma_start(wpx_sb[:], d_wpx[:])
        wpy_sb = cst.tile([D, D], f16, tag="wpy"); nc.sync.dma_start(wpy_sb[:], d_wpy[:])
        id32_sb = cst.tile([D, D], f32, tag="id32"); nc.sync.dma_start(id32_sb[:], d_id32[:])

        # bn_stats over a [P, 2, 128] chunk-pair computes per-chunk stats in
        # its even/odd halves directly (no merge): col1/col4 = chunk means,
        # col2/col5 = chunk M2s. rstd = newton-corrected 1/sqrt(M2/n + eps).
        def stats_rstd(s, P, NPAIRS):
            n = 128
            vv = st.tile([P, NPAIRS, 2], f32, tag=f"vv{P}{NPAIRS}")
            nc.gpsimd.tensor_scalar(out=vv[:], in0=s[:P, :, 2:6:3], scalar1=1.0 / float(n),
                                    scalar2=float(LN_EPS), op0=MUL, op1=ADD)
            rstd = st.tile([P, NPAIRS, 2], f32, tag=f"rstd{P}{NPAIRS}")
            nc.scalar.activation(rstd[:], vv[:], mybir.ActivationFunctionType.Sqrt)
            nc.vector.reciprocal(rstd[:], rstd[:])
            t2 = st.tile([P, NPAIRS, 2], f32, tag=f"t2{P}{NPAIRS}")
            eng.tensor_tensor(out=t2[:], in0=rstd[:], in1=rstd[:], op=MUL)
            eng.tensor_tensor(out=t2[:], in0=t2[:], in1=vv[:], op=MUL)
            eng.tensor_scalar(out=t2[:], in0=t2[:], scalar1=-0.5, scalar2=1.5, op0=MUL, op1=ADD)
            eng.tensor_tensor(out=rstd[:], in0=rstd[:], in1=t2[:], op=MUL)
            return rstd

        pair_tiles = {}

        def emit_load_pair(j):
            xq = xp.tile([128, 2 * QC, D], f16, tag="xq", name=f"xq{j}")
            nc.sync.dma_start(xq[:], d_q[2 * j:2 * j + 2].rearrange("l (c p) d -> p (l c) d", p=128))
            xk = xp.tile([KCS, 2 * KC, D], f16, tag="xk", name=f"xk{j}")
            nc.sync.dma_start(xk[:], d_k[2 * j:2 * j + 2].rearrange("l (c p) d -> p (l c) d", p=KCS))
            xv = xp.tile([KCS, 2 * KC, D], f16, tag="xv", name=f"xv{j}")
            nc.sync.dma_start(xv[:], d_v[2 * j:2 * j + 2].rearrange("l (c p) d -> p (l c) d", p=KCS))
            pair_tiles[j] = dict(xq=xq, xk=xk, xv=xv)

        def emit_load_skip(j):
            skp = outp.tile([64, 2, D], f32, tag="skp", name=f"skp{j}")
            nc.sync.dma_start(skp[:], d_skip[2 * j:2 * j + 2].rearrange("l t d -> t l d"))
            pair_tiles[j]["skp"] = skp

        emit_load_pair(0)

        # constants (transpose identity + projection weights first: they gate
        # the first PE work; the rest can trickle in behind pair-0 stats)
        idh_sb = cst.tile([D, D], f16, tag="idh"); nc.sync.dma_start(idh_sb[:], d_idh[:])
        wq_sb = cst.tile([D, D], f16, tag="wq"); nc.sync.dma_start(wq_sb[:], d_wq[:])
        wk_sb = cst.tile([D, D], f16, tag="wk"); nc.sync.dma_start(wk_sb[:], d_wk[:])
        wv_sb = cst.tile([D, D], f16, tag="wv"); nc.sync.dma_start(wv_sb[:], d_wv[:])
        cq_sb = cst.tile([D, 1], f32, tag="cq"); nc.sync.dma_start(cq_sb[:], d_cq[:])
        wpx_sb = cst.tile([D, D], f16, tag="wpx"); nc.sync.dma_start(wpx_sb[:], d_wpx[:])
        wpy_sb = cst.tile([D, D], f16, tag="wpy"); nc.sync.dma_start(wpy_sb[:], d_wpy[:])
        id32_sb = cst.tile([D, D], f32, tag="id32"); nc.sync.dma_start(id32_sb[:], d_id32[:])

        # per-pair even/odd bn_stats combine -> mu, rstd  (chunk width 128)
        def stats_combine(s, P, NCH, eng=None):
            eng = eng or nc.gpsimd
            n = 128
            me, mo = s[:P, :, 1:2].rearrange("p c o -> p (c o)"), s[:P, :, 4:5].rearrange("p c o -> p (c o)")
            m2e, m2o = s[:P, :, 2:3].rearrange("p c o -> p (c o)"), s[:P, :, 5:6].rearrange("p c o -> p (c o)")
            mu = st.tile([P, NCH], f32, tag=f"mu{P}{NCH}")
            rstd = st.tile([P, NCH], f32, tag=f"rstd{P}{NCH}")
            tmp = st.tile([P, NCH], f32, tag=f"tmp{P}{NCH}")
            m2 = st.tile([P, NCH], f32, tag=f"m2{P}{NCH}")
            eng.tensor_tensor(out=tmp[:], in0=me, in1=mo, op=ADD)
            eng.tensor_scalar_mul(mu[:], tmp[:], 0.5)
            eng.tensor_tensor(out=tmp[:], in0=mo, in1=me, op=SUB)
            eng.tensor_tensor(out=tmp[:], in0=tmp[:], in1=tmp[:], op=MUL)
            eng.tensor_tensor(out=m2[:], in0=m2e, in1=m2o, op=ADD)
            nc.gpsimd.scalar_tensor_tensor(out=m2[:], in0=tmp[:], scalar=float(n) / 4.0, in1=m2[:], op0=MUL, op1=ADD)
            # v = var + eps ; r0 = 1/sqrt_approx(v) ; one Newton step for the
            # loose ACT sqrt: r1 = r0*(1.5 - 0.5*v*r0^2)
            vv = st.tile([P, NCH], f32, tag=f"vv{P}{NCH}")
            eng.tensor_scalar(out=vv[:], in0=m2[:], scalar1=1.0 / float(n),
                                    scalar2=float(LN_EPS), op0=MUL, op1=ADD)
            nc.scalar.activation(rstd[:], vv[:], mybir.ActivationFunctionType.Sqrt)
            nc.vector.reciprocal(rstd[:], rstd[:])
            t2 = st.tile([P, NCH], f32, tag=f"t2{P}{NCH}")
            eng.tensor_tensor(out=t2[:], in0=rstd[:], in1=rstd[:], op=MUL)
            eng.tensor_tensor(out=t2[:], in0=t2[:], in1=vv[:], op=MUL)
            eng.tensor_scalar(out=t2[:], in0=t2[:], scalar1=-0.5, scalar2=1.5, op0=MUL, op1=ADD)
            eng.tensor_tensor(out=rstd[:], in0=rstd[:], in1=t2[:], op=MUL)
            return mu, rstd

        def emit_stats_q(j, eng=None):
            # stats + LN apply for both groups of the pair (LN on Pool in
            # steady state; the prologue routes through DVE for latency)
            eng = eng or nc.gpsimd
            pt = pair_tiles[j]
            qs = st.tile([128, 2 * QC, 6], f32, tag="qs", name=f"qs{j}")
            for i in range(2 * QC):
                nc.vector.bn_stats(qs[:, i, :], pt["xq"][:, i, :])
            qmu, qrstd = stats_combine(qs, 128, 2 * QC, eng)
            for i in range(2 * QC):
                eng.tensor_scalar(out=pt["xq"][:, i, :], in0=pt["xq"][:, i, :],
                                  scalar1=qmu[:, i:i + 1], scalar2=qrstd[:, i:i + 1],
                                  op0=SUB, op1=MUL)

        def emit_stats_kv(j, eng=None):
            # LN on Pool (steady state) / DVE (prologue); k before v so the
            # score path unblocks first
            eng = eng or nc.gpsimd
            pt = pair_tiles[j]
            ks = st.tile([KCS, 4 * KC, 6], f32, tag="ks", name=f"ks{j}")
            for i in range(2 * KC):
                nc.vector.bn_stats(ks[:, i, :], pt["xk"][:, i, :])
            for i in range(2 * KC):
                nc.vector.bn_stats(ks[:, 2 * KC + i, :], pt["xv"][:, i, :])
            kmu, krstd = stats_combine(ks, KCS, 4 * KC, eng)
            for x, base in ((pt["xk"], 0), (pt["xv"], 2 * KC)):
                for i in range(2 * KC):
                    eng.tensor_scalar(out=x[:, i, :], in0=x[:, i, :],
                                      scalar1=kmu[:, base + i:base + i + 1],
                                      scalar2=krstd[:, base + i:base + i + 1],
                                      op0=SUB, op1=MUL)

        group_state = {}

        class _ActDrain:
            tensor_copy = staticmethod(lambda out, in_: nc.scalar.copy(out, in_))

        def drain_eng(l):
            # ACT is idle during the prologue; afterwards DVE takes the drains
            return _ActDrain if l < 2 else nc.vector

        def prep_thunks(l):
            """Pre-attention work for group l as a list of emission thunks."""
            j, g = l // 2, l % 2
            gs = group_state.setdefault(l, {})

            def tr_q():
                pt = pair_tiles[j]
                tp_q = work.tile([128, 1024], f16, tag="w", name=f"tpq{l}")
                for c in range(QC):
                    nc.tensor.transpose(tp_q[:, c * 128:(c + 1) * 128], pt["xq"][:, g * QC + c, :], idh_sb[:])
                gs["xqT"] = sb.tile([128, TQ], f16, tag="xqT", name=f"xqT{l}")
                drain_eng(l).tensor_copy(gs["xqT"][:], tp_q[:, 0:TQ])

            def tr_k():
                pt = pair_tiles[j]
                tp_k = work.tile([128, 1024], f16, tag="w", name=f"tpk{l}")
                for c in range(KC):
                    nc.tensor.transpose(tp_k[:, c * KCS:(c + 1) * KCS], pt["xk"][:, g * KC + c, :],
                                        idh_sb[0:KCS, 0:KCS])
                gs["xkT"] = sb.tile([128, TK], f16, tag="xkT", name=f"xkT{l}")
                drain_eng(l).tensor_copy(gs["xkT"][:], tp_k[:, 0:TK])

            def tr_v():
                pt = pair_tiles[j]
                tp_v = work.tile([128, 1024], f16, tag="w", name=f"tpv{l}")
                for c in range(KC):
                    nc.tensor.transpose(tp_v[:, c * KCS:(c + 1) * KCS], pt["xv"][:, g * KC + c, :],
                                        idh_sb[0:KCS, 0:KCS])
                gs["xvT"] = sb.tile([128, TK], f16, tag="xvT", name=f"xvT{l}")
                drain_eng(l).tensor_copy(gs["xvT"][:], tp_v[:, 0:TK])

            def proj_q():
                qt_ps = work.tile([128, 512], f32, tag="w", name=f"qtp{l}")
                nc.tensor.matmul(qt_ps[:, 0:TQ], wq_sb[:], gs["xqT"][:], start=True, stop=True)
                gs["QT"] = sb.tile([128, TQ], f16, tag="QT", name=f"QT{l}")
                nc.vector.tensor_scalar(out=gs["QT"][:], in0=qt_ps[:, 0:TQ],
                                        scalar1=cq_sb[:, 0:1], scalar2=None, op0=ADD)

            def proj_k():
                kt_ps = work.tile([128, 512], f32, tag="w", name=f"ktp{l}")
                nc.tensor.matmul(kt_ps[:, 0:TK], wk_sb[:], gs["xkT"][:], start=True, stop=True)
                gs["KT"] = sb.tile([128, TK], f16, tag="KT", name=f"KT{l}")
                drain_eng(l).tensor_copy(gs["KT"][:], kt_ps[:, 0:TK])

            def proj_v():
                v_ps = work.tile([128, 512], f32, tag="w", name=f"vp{l}")
                for c in range(KC):
                    nc.tensor.matmul(v_ps[0:KCS, c * 128:(c + 1) * 128].rearrange("p f -> p f"),
                                     gs["xvT"][:, c * KCS:(c + 1) * KCS], wv_sb[:], start=True, stop=True)
                gs["Vaug"] = sb.tile([KCS, KC, 4, 64], bf16, tag="Vaug", name=f"Vaug{l}")
                nc.scalar.copy(gs["Vaug"][:, :, :, 0:32],
                               v_ps[0:KCS, 0:512].rearrange("p (c h d) -> p c h d", h=4, d=32))
                nc.gpsimd.memset(gs["Vaug"][:, :, :, 32:64], 1.0)

            return [tr_q, proj_q, tr_k, proj_k, tr_v, proj_v]

        def run_all(thunks):
            for t in thunks:
                t()

        emit_load_pair(1)
        th0 = prep_thunks(0)  # [tr_q, proj_q, tr_k, proj_k, tr_v, proj_v]
        emit_stats_q(0)
        run_all(th0[0:2])
        emit_stats_kv(0)
        run_all(th0[2:])
        emit_load_skip(0)

        all_units = [(l, h, p2) for l in range(LPC) for h in range(HEADS) for p2 in range(2)]
        sc_map = {}
        at_map = {}
        atx_map = {}
        inter_state = {"q": [], "ui": 0}

        def emit_scores(u):
            l, h, p2 = u
            gs = group_state[l]
            hs = h * 32
            sc = scb.tile([128, 2, 512], f32, tag="sc", name=f"sc{l}{h}{p2}")
            for c2 in range(2):
                c = 2 * p2 + c2
                nc.tensor.matmul(sc[0:KCS, c2, 0:TQ], gs["KT"][hs:hs + 32, c * KCS:(c + 1) * KCS],
                                 gs["QT"][hs:hs + 32, :], start=True, stop=True, tile_position=(hs, 0))
            sc_map[u] = sc

        def emit_exp(u):
            l, h, p2 = u
            sc = sc_map.pop(u)
            pt = ptp.tile([KCS, 2, TQ], bf16, tag="pt", name=f"pt{l}{h}{p2}")
            nc.scalar.activation(pt[:], sc[0:KCS, :, 0:TQ], mybir.ActivationFunctionType.Exp)
            return pt

        def emit_avsu(u, pt):
            l, h, p2 = u
            gs = group_state[l]
            bank = at_map[l][h // 2]
            hs2 = 64 * (h % 2)
            for c2 in range(2):
                c = 2 * p2 + c2
                nc.tensor.matmul(bank[hs2:hs2 + 64, 0:TQ], gs["Vaug"][:, c, h, :], pt[:, c2, :],
                                 start=(c == 0), stop=(c == KC - 1),
                                 tile_position=(0, hs2), skip_group_check=True)

        def emit_division(l, b):
            # bank rows per head pair: [at_h (0:32) | sum_h (32:64) | at_h' | sum_h']
            # recip of the whole bank: at-row reciprocals are unused garbage.
            # att/sum division: out aligned with in1 (sbuf); in0 is the
            # misaligned psum operand (allowed cross-space). rows 0:32 / 64:96
            # of AT are never written; their wp rows are zero, but
            # 0*garbage-NaN would poison - keep them zeroed (Pool)
            bank = at_map[l][b]
            tag = "XY"[b]
            R = sb.tile([128, TQ], f32, tag=f"R{tag}", name=f"R{tag}{l}")
            nc.vector.reciprocal(R[:], bank[:, 0:TQ])
            AT = sb.tile([128, TQ], f16, tag=f"AT{tag}", name=f"AT{tag}{l}")
            # rows never written by the divisions; their wp rows are zero, but
            # 0*garbage-NaN would poison - keep them zeroed (Pool)
            nc.gpsimd.memset(AT[0:32, :], 0.0)
            nc.gpsimd.memset(AT[64:96, :], 0.0)
            nc.vector.tensor_tensor(out=AT[32:64, :], in0=bank[0:32, 0:TQ], in1=R[32:64, :], op=MUL)
            nc.vector.tensor_tensor(out=AT[96:128, :], in0=bank[64:96, 0:TQ], in1=R[96:128, :], op=MUL)
            return AT

        def emit_tail(l, ATX):
            j, g = l // 2, l % 2
            ATY = emit_division(l, 1)
            at_map.pop(l)
            zt_ps = work.tile([128, 512], f32, tag="w", name=f"zt{l}")
            nc.tensor.matmul(zt_ps[:, 0:TQ], wpx_sb[:], ATX[:], start=True, stop=False, skip_group_check=True)
            nc.tensor.matmul(zt_ps[:, 0:TQ], wpy_sb[:], ATY[:], start=False, stop=True, skip_group_check=True)
            zm = sb.tile([128, 64], f32, tag="zm", name=f"zm{l}")
            nc.vector.reduce_sum(zm[:], zt_ps[:, 0:TQ].rearrange("p (n w) -> p w n", n=6), axis=X)
            zt2 = work.tile([128, 512], f32, tag="w", name=f"zt2{l}")
            nc.tensor.transpose(zt2[0:64, 0:128], zm[:], id32_sb[:])
            if g == 0:
                pair_tiles[j]["o_pr"] = outp.tile([64, 2, D], f32, tag="o", name=f"o{j}")
            o_pr = pair_tiles[j]["o_pr"]
            nc.vector.tensor_tensor(out=o_pr[:, g, :], in0=zt2[0:64, 0:128],
                                    in1=pair_tiles[j]["skp"][:, g, :], op=ADD)
            if g == 1:
                nc.sync.dma_start(d_out[l - 1:l + 1].rearrange("l t d -> t l d"), o_pr[:])

        def start_group(l):
            j, g = l // 2, l % 2
            # heads 0,1 accumulate [att | row-sums] into augX (64 rows each),
            # heads 2,3 into augY; per-head start_tensor_calc zeroes only that
            # head's own 64-partition band of the bank
            augx = asb.tile([128, 512], f32, tag="agx", name=f"agx{l}")
            augy = asb.tile([128, 512], f32, tag="agy", name=f"agy{l}")
            at_map[l] = (augx, augy)
            inter = []
            if g == 0:
                if l + 4 < LPC:
                    inter.append(lambda jj=j + 2: emit_load_pair(jj))
                if l + 2 < LPC:
                    inter.append(lambda jj=j + 1: emit_stats_q(jj))
                    inter.append(lambda jj=j + 1: emit_stats_kv(jj))
            if l + 1 < LPC:
                inter.extend(prep_thunks(l + 1))
            if g == 0 and l + 2 < LPC:
                inter.append(lambda jj=j + 1: emit_load_skip(jj))
            inter_state["q"] = inter
            inter_state["ui"] = 0

        start_group(0)
        emit_scores(all_units[0])
        for idx, u in enumerate(all_units):
            l = u[0]
            pt = emit_exp(u)
            if idx + 1 < len(all_units):
                nu = all_units[idx + 1]
                if nu[0] != l:
                    start_group(nu[0])
                emit_scores(nu)
            # interleave prep between scores(u+1) and at/su(u): PE would
            # otherwise stall in-order on exp(u) with ready work behind it
            for _ in range(2):
                if inter_state["ui"] < len(inter_state["q"]):
                    inter_state["q"][inter_state["ui"]]()
                    inter_state["ui"] += 1
            emit_avsu(u, pt)
            if u[1] == 1 and u[2] == 1:
                atx_map[l] = emit_division(l, 0)
            if u[1] == HEADS - 1 and u[2] == 1:
                while inter_state["ui"] < len(inter_state["q"]):
                    inter_state["q"][inter_state["ui"]]()
                    inter_state["ui"] += 1
                emit_tail(l, atx_map.pop(l))

    # -- walrus workaround #2: this build rejects >1 sync-wait per instruction;
    # hoist excess waits onto single-wait NOPs just before each instruction --
    for fn in nc.m.functions:
        for blk in fn.blocks:
            insts = blk.instructions
            i = 0
            while i < len(insts):
                inst = insts[i]
                si = getattr(inst, "sync_info", None)
                waits = list(si.on_wait) if si is not None else []
                if len(waits) > 1:
                    inst.sync_info = mybir.SyncInfo(on_wait=waits[-1:], on_update=list(si.on_update))
                    for jj, w in enumerate(waits[:-1]):
                        nop = mybir.InstNoOp(name=f"{inst.name}-sw{jj}", ins=[], outs=[])
                        nop.engine = inst.engine
                        nop.sync_info = mybir.SyncInfo(on_wait=[w], on_update=[])
                        insts.insert(i, nop)
                        i += 1
                i += 1
    return nc


def _get_prog():
    if "nc" not in _prog:
        _prog["nc"] = _build_program()
    return _prog["nc"]


def kernel(q, k, v, skip, attn_scale, lnq_g, lnq_b, Wq, bq, lnk_g, lnk_b, Wk, bk,
           lnv_g, lnv_b, Wv, bv, Wp, bp):
    from concourse.bass_utils import run_bass_kernel_spmd
    import ml_dtypes

    bf = ml_dtypes.bfloat16
    q = np.asarray(q); k = np.asarray(k); v = np.asarray(v); skip = np.asarray(skip)
    b, n, Xd, Yd, W1, W2, d = q.shape
    _, _, x2, y2, w1, w2, _ = k.shape
    L = Xd * Yd
    # rearrange to [L, tokens, d] token-major (token = n*W1*W2 + w1*W2 + w2)
    qf = np.ascontiguousarray(q.transpose(0, 2, 3, 1, 4, 5, 6).reshape(L, n * W1 * W2, d)).astype(np.float16)
    kf = np.ascontiguousarray(k.transpose(0, 2, 3, 1, 4, 5, 6).reshape(L, n * w1 * w2, d)).astype(np.float16)
    vf = np.ascontiguousarray(v.transpose(0, 2, 3, 1, 4, 5, 6).reshape(L, n * w1 * w2, d)).astype(np.float16)
    # cv (the folded V bias) adds a constant to every attention output; its
    # post-projection effect is cv @ Wp, constant across tokens/views -> fold
    # into the skip. ck shifts all scores of a head equally -> softmax
    # invariant, dropped.
    cv_vec = (np.asarray(lnv_b) @ np.asarray(Wv) + np.asarray(bv)).astype(np.float64)
    cv_z = (cv_vec @ np.asarray(Wp).astype(np.float64)).astype(np.float32)
    sf = np.ascontiguousarray(skip.reshape(L, W1 * W2, d) + bp[None, None, :]
                              + cv_z[None, None, :]).astype(np.float32)

    s_tot = float(np.asarray(attn_scale).reshape(-1)[0]) * (DH ** -0.5)
    wq_eff = (np.asarray(lnq_g)[:, None] * np.asarray(Wq) * s_tot).astype(np.float16)
    cq = (np.asarray(lnq_b) @ np.asarray(Wq) * s_tot + np.asarray(bq) * s_tot).astype(np.float32).reshape(D, 1)
    wk_eff = (np.asarray(lnk_g)[:, None] * np.asarray(Wk)).astype(np.float16)
    wv_eff = (np.asarray(lnv_g)[:, None] * np.asarray(Wv)).astype(np.float16)
    wp_eff = (np.asarray(Wp) / float(n)).astype(np.float16)
    # out-projection split for the [att | row-sum] psum bank layout: valid AT
    # rows are 32:64 (first head of the pair) and 96:128 (second head)
    wpx = np.zeros((D, D), np.float16)
    wpy = np.zeros((D, D), np.float16)
    wpx[32:64] = wp_eff[0:32]; wpx[96:128] = wp_eff[32:64]
    wpy[32:64] = wp_eff[64:96]; wpy[96:128] = wp_eff[96:128]
    identh = np.eye(D, dtype=np.float16)
    ident32 = np.eye(D, dtype=np.float32)

    nc = _get_prog()
    shared = dict(wq=wq_eff, wk=wk_eff, wv=wv_eff, wpx=wpx, wpy=wpy, cq=cq,
                  identh=identh, ident32=ident32)
    in_maps = []
    for c in range(NCORES):
        s = slice(c * LPC, (c + 1) * LPC)
        in_maps.append(dict(q=np.ascontiguousarray(qf[s]), k=np.ascontiguousarray(kf[s]),
                            v=np.ascontiguousarray(vf[s]), skip=np.ascontiguousarray(sf[s]),
                            **shared))
    res = run_bass_kernel_spmd(nc, in_maps, core_ids=list(range(NCORES)))
    kernel._last_results = res
    outs = np.concatenate([res.results[c]["out"] for c in range(NCORES)], axis=0)  # [64, 64, 128]
    return outs.reshape(Xd, Yd, W1, W2, d)[None].astype(np.float32)
